# revision 5
# baseline (speedup 1.0000x reference)
"""Trainium2 Bass kernel for nn_Attention_18786186952997.

Dense causal-attention transformer block with ternarized (BitNet-style)
weights and RoPE:

    wq = ternarize(w_qkv); wp = ternarize(w_proj)
    qkv = x @ wq.T ; q,k,v split ; RoPE(q,k) ; causal SDPA ; y @ wp.T

Sharding: 8 cores = 2 batches x 4 head-groups (4 heads each).  Each core
computes its batch's qkv projections for its 4 heads, runs causal
flash-style attention fully on-chip, and produces a partial (transposed)
projection output; the host sums the 4 partials per batch (scaled by
am_q*am_p on the host, so the device works with pure sign weights).

Device compute layout is channel-major: q.T/k.T are produced as
[head_dim, tokens].  The head-dim rows are interleaved (d, d+32) pairs so
RoPE's rotate-half becomes a swap-adjacent-rows stream_shuffle (one DVE
op) instead of four 32-row shifted multiplies.  exp(scores.T) is exactly
the stationary layout A@V needs; softmax denominators come free from
ones-columns packed next to V.

q/k projections run on single-precision fp8 x with DoubleRow packing a
PAIR of 128-channel contraction chunks per matmul (half the matmuls of
the residual-pair scheme; ~7e-3 rel err, sim-validated).  v projections
run on bf16 x without DoubleRow so FWL keeps LDWEIGHTS off the critical
path.  Scores/AV/proj stay bf16.  Dummy matmuls at t=0 warm the PE HAM
clock gate while the first DMAs land.  Causal masking affine_selects
only the 128-column diagonal band.  Phases run in causal order with
independent PE work spliced between attention phases.
"""

import os
import sys
import types

import numpy as np

sys.path.insert(0, "/opt/trn_rl_repo")

import ml_dtypes  # noqa: E402

BF16 = ml_dtypes.bfloat16
F8E4 = ml_dtypes.float8_e4m3

B, T, C, H, D = 2, 2048, 1024, 16, 64
N_CORES = 8
HEADS_PER_CORE = 4
P = 128
QT = 512            # q tile (moving free dim)
NQT = T // QT       # 4
NKC = T // P        # 16 k chunks
NCC = C // P        # 8 contraction chunks

_CACHE = {}


def _install_ntff_hook():
    """bass_utils' trace=True path needs antenv.axon_hooks, absent in this
    image; synthesize it around the boot module's ctypes hook."""
    if "antenv.axon_hooks" in sys.modules:
        return
    try:
        import antenv  # noqa: F401
        from trn_agent_boot.trn_boot import _ntff_profile_via_ctypes
    except Exception:
        return
    mod = types.ModuleType("antenv.axon_hooks")
    holder = {}
    mod.set_axon_ntff_profile_hook = lambda h: holder.__setitem__("h", h)
    mod.get_axon_ntff_profile_hook = lambda: holder.get("h")
    sys.modules["antenv.axon_hooks"] = mod
    sys.modules["antenv"].axon_hooks = mod
    try:
        hook = _ntff_profile_via_ctypes("/opt/axon/libaxon_pjrt.so")
        mod.set_axon_ntff_profile_hook(hook)
    except Exception:
        pass


def _ternarize_host(w):
    """Sign matrix and abs-mean scale, bit-matching the jax reference."""
    try:
        import jax.numpy as jnp

        wj = jnp.asarray(w)
        am = jnp.maximum(jnp.abs(wj).mean(), 1e-5)
        thr = 0.7 * am
        s = jnp.where(wj > thr, 1.0, jnp.where(wj < -thr, -1.0, 0.0))
        return np.asarray(s, dtype=np.float32), np.float32(am)
    except Exception:
        am = np.float32(max(np.abs(w).astype(np.float32).mean(dtype=np.float32), 1e-5))
        thr = np.float32(0.7) * am
        s = np.where(w > thr, 1.0, np.where(w < -thr, -1.0, 0.0)).astype(np.float32)
        return s, am


def _build_program(exp_scale):
    import concourse.bass as bass  # noqa: F401
    import concourse.mybir as mybir
    import concourse.tile as tile
    from concourse import bacc

    F32 = mybir.dt.float32
    BF = mybir.dt.bfloat16
    F8 = mybir.dt.float8e4
    AF = mybir.ActivationFunctionType
    GE = mybir.AluOpType.is_ge
    DR = mybir.MatmulPerfMode.DoubleRow

    nc = bacc.Bacc("TRN2", target_bir_lowering=False, debug=False,
                   num_devices=N_CORES)

    # fp8 single-precision x for q/k (DoubleRow packs kc pairs); bf16 x
    # for v; ternary weights exact in fp8/bf16
    xb = nc.dram_tensor("xb", [C, T], BF, kind="ExternalInput").ap()
    xq8 = nc.dram_tensor("xq8", [C, T], F8, kind="ExternalInput").ap()
    wqk = nc.dram_tensor("wqk", [C, 512], F8, kind="ExternalInput").ap()
    wv = nc.dram_tensor("wv", [C, 256], BF, kind="ExternalInput").ap()
    wp = nc.dram_tensor("wp", [256, 1024], BF, kind="ExternalInput").ap()
    csss = nc.dram_tensor("csss", [P, 2, T], BF, kind="ExternalInput").ap()
    outT = nc.dram_tensor("outT", [C, T], BF, kind="ExternalOutput").ap()

    # swap-adjacent-rows shuffle mask (per 32-partition quadrant)
    swap_mask = []
    for i in range(16):
        swap_mask += [2 * i + 1, 2 * i]

    with tile.TileContext(nc) as tc:
        with (
            tc.tile_pool(name="consts", bufs=1) as consts,
            tc.tile_pool(name="tmps", bufs=3) as tmps,
            tc.tile_pool(name="epool", bufs=6) as epool,
            tc.tile_pool(name="opool", bufs=2) as opool,
            tc.tile_pool(name="ps_big", bufs=3, space="PSUM") as ps_big,
            tc.tile_pool(name="ps_y", bufs=2, space="PSUM") as ps_y,
        ):
            # ---- persistent SBUF allocations ----
            cs_sb = consts.tile([P, 2, T], BF)          # cos | signed-sin
            x_sb = consts.tile([P, NCC, T], BF)         # bf16 x (for v)
            xq_sb = consts.tile([P, 4, 2, T], F8)       # fp8 x (for q/k)
            wqk_sb = consts.tile([P, 4, 2, 512], F8)
            wv_sb = consts.tile([P, NCC, 256], BF)
            wp_sb = consts.tile([P, 2, 1024], BF)
            qk_sb = consts.tile([P, 4, T], BF)  # blk: q01, q23, k01, k23
            v_sb = consts.tile([P, NKC, 2, 256], BF)
            y_sb = consts.tile([P, 2, T], BF)
            warm_sb = consts.tile([P, 512], BF)

            xb_p = xb.rearrange("(n p) t -> p n t", p=P)
            xq_p = xq8.rearrange("(n r p) t -> p n r t", p=P, r=2)
            wqk_p = wqk.rearrange("(n r p) m -> p n r m", p=P, r=2)
            wv_p = wv.rearrange("(n p) m -> p n m", p=P)
            wp_p = wp.rearrange("(n p) m -> p n m", p=P)
            outT_p = outT.rearrange("(m p) t -> p m t", p=P)

            # ---- PE warm-up: ~4us of dummy matmuls releases the HAM
            # clock gate while the first input DMAs are still landing ----
            nc.vector.memset(warm_sb, 0.0)
            wps = ps_y.tile([P, 512], F32, tag="y", name="warm")
            for i in range(9):
                nc.tensor.matmul(wps, lhsT=warm_sb[:, 0:P], rhs=warm_sb,
                                 start=(i == 0), stop=(i == 8))

            # ---- DMA-in, ordered by first use; three queues run in
            # parallel (sync: fp8 x, gpsimd: bf16 x, scalar: weights) ----
            nc.sync.dma_start(out=xq_sb[:, :, :, 0:QT],
                              in_=xq_p[:, :, :, 0:QT])
            nc.scalar.dma_start(out=wqk_sb, in_=wqk_p)
            nc.scalar.dma_start(out=cs_sb[:, :, 0:QT], in_=csss[:, :, 0:QT])
            nc.gpsimd.dma_start(out=x_sb[:, :, 0:QT], in_=xb_p[:, :, 0:QT])
            nc.scalar.dma_start(out=wv_sb, in_=wv_p)
            for qt in range(1, NQT):
                qs = slice(qt * QT, (qt + 1) * QT)
                nc.sync.dma_start(out=xq_sb[:, :, :, qs], in_=xq_p[:, :, :, qs])
                nc.gpsimd.dma_start(out=x_sb[:, :, qs], in_=xb_p[:, :, qs])
            nc.scalar.dma_start(out=cs_sb[:, :, QT:], in_=csss[:, :, QT:])
            nc.scalar.dma_start(out=wp_sb, in_=wp_p)

            # per head: [ones(64) | v(64)] -> denominators at psum rows 0:64
            v_sb4 = v_sb.rearrange("p n g (h o d) -> p n g h o d", h=2, o=2)
            nc.gpsimd.memset(v_sb4[:, :, :, :, 0, :], 1.0)

            def emit_qkv(qt):
                # wqk col blocks: QA[0:256) KA[256:512)
                qs = slice(qt * QT, (qt + 1) * QT)
                for pair in range(2):
                    blks = ((0, 2), (1, 3))[pair]
                    ps = ps_big.tile([P, 1024], F32, tag="big", name="qkvps")
                    for kcp in range(4):
                        for j, blk in enumerate(blks):
                            base_a = [0, 128, 256, 384][blk]
                            nc.tensor.matmul(
                                ps[:, j * QT:(j + 1) * QT],
                                lhsT=wqk_sb[:, kcp, :, base_a:base_a + P],
                                rhs=xq_sb[:, kcp, :, qs],
                                start=(kcp == 0),
                                stop=(kcp == 3),
                                perf_mode=DR,
                            )
                    for j, blk in enumerate(blks):
                        pj = ps[:, j * QT:(j + 1) * QT]
                        # rows interleaved (d, d+32): rotate-half = swap pairs
                        t2s = tmps.tile([P, QT], F32, tag="t2s")
                        nc.vector.stream_shuffle(t2s, pj, swap_mask)
                        t1 = tmps.tile([P, QT], F32, tag="t1")
                        nc.vector.tensor_mul(t1, pj, cs_sb[:, 0, qs])
                        t2 = tmps.tile([P, QT], F32, tag="t2")
                        nc.gpsimd.tensor_mul(t2, t2s, cs_sb[:, 1, qs])
                        nc.gpsimd.tensor_add(qk_sb[:, blk, qs], t1, t2)

            def emit_v(tt):
                vp = ps_y.tile([P, 512], F32, tag="y", name="vps")
                for kc in range(NCC):
                    nc.tensor.matmul(
                        vp[:, 0:256],
                        lhsT=x_sb[:, kc, tt * P:(tt + 1) * P],
                        rhs=wv_sb[:, kc, :],
                        start=(kc == 0),
                        stop=(kc == NCC - 1),
                    )
                vp4 = vp[:, 0:256].rearrange("p (g h d) -> p g h d", g=2, h=2)
                nc.vector.tensor_copy(v_sb4[:, tt, :, :, 1, :], vp4)

            def emit_attn(grp, qt):
                q_t = qk_sb[:, grp, :]
                k_t = qk_sb[:, 2 + grp, :]
                qs = slice(qt * QT, (qt + 1) * QT)
                KC = 4 * (qt + 1)  # causal k chunks
                yA = ps_y.tile([P, QT], F32, tag="y", name="yA")
                yB = ps_y.tile([P, QT], F32, tag="y", name="yB")
                es = [None] * KC

                def emit_sc(kc):
                    ks = slice(kc * P, (kc + 1) * P)
                    delta = max(kc * P - qt * QT, 0)
                    # queries < delta cannot see this key chunk: compute
                    # scores/exp only on the [delta:QT) query slice
                    qsl = slice(qt * QT + delta, (qt + 1) * QT)
                    ps = ps_big.tile([P, 1024], F32, tag="big", name="scps")
                    p2 = ps.rearrange("p (j f) -> p j f", j=2)
                    e = epool.tile([P, 1024], BF, tag="e")
                    e2 = e.rearrange("p (j f) -> p j f", j=2)
                    nc.tensor.matmul(p2[:, 0, delta:QT], lhsT=k_t[0:64, ks],
                                     rhs=q_t[0:64, qsl],
                                     start=True, stop=True)
                    nc.tensor.matmul(p2[:, 1, delta:QT], lhsT=k_t[64:128, ks],
                                     rhs=q_t[64:128, qsl],
                                     start=True, stop=True)
                    nc.scalar.activation(e2[:, :, delta:QT],
                                         p2[:, :, delta:QT],
                                         AF.Exp, scale=exp_scale)
                    if kc * P >= qt * QT:
                        # diagonal chunk: zero keys below the diagonal for
                        # both heads in one op (iota = col' - p >= 0); only
                        # the first 128 query columns past delta are mixed
                        nc.gpsimd.affine_select(
                            e2[:, :, delta:delta + P], e2[:, :, delta:delta + P],
                            pattern=[[0, 2], [1, P]],
                            compare_op=GE, fill=0.0,
                            base=0, channel_multiplier=-1)
                    es[kc] = (e2, delta)

                def emit_av(kc):
                    e2, delta = es[kc]
                    nc.tensor.matmul(yA[:, delta:QT],
                                     lhsT=v_sb[:, kc, grp, 0:128],
                                     rhs=e2[:, 0, delta:QT],
                                     start=(kc == 0), stop=(kc == KC - 1),
                                     skip_group_check=True)
                    nc.tensor.matmul(yB[:, delta:QT],
                                     lhsT=v_sb[:, kc, grp, 128:256],
                                     rhs=e2[:, 1, delta:QT],
                                     start=(kc == 0), stop=(kc == KC - 1),
                                     skip_group_check=True)

                # depth-4 software pipeline: AV lags scores by 4 chunks
                LAG = min(4, KC - 1)
                for kc in range(LAG):
                    emit_sc(kc)
                for kc in range(LAG, KC):
                    emit_sc(kc)
                    emit_av(kc - LAG)
                for kc in range(KC - LAG, KC):
                    emit_av(kc)

                # both heads: denom rows 0:64, y rows 64:128
                rcA = tmps.tile([P, QT], F32, tag="rc")
                nc.vector.reciprocal_approx_fast(rcA[0:64, :], yA[0:64, :])
                nc.vector.tensor_mul(y_sb[0:64, grp, qs], yA[64:128, :],
                                     rcA[0:64, :])
                rcB = tmps.tile([P, QT], F32, tag="rc")
                nc.vector.reciprocal_approx_fast(rcB[0:64, :], yB[0:64, :])
                nc.vector.tensor_mul(y_sb[64:128, grp, qs], yB[64:128, :],
                                     rcB[0:64, :])

            def emit_proj(qt):
                qs = slice(qt * QT, (qt + 1) * QT)
                ot = opool.tile([P, 8, QT], BF, tag="ot")
                for mt in range(8):
                    ms = slice(mt * P, (mt + 1) * P)
                    pp = ps_y.tile([P, QT], F32, tag="y", name="pp")
                    for ch in range(2):
                        nc.tensor.matmul(pp, lhsT=wp_sb[:, ch, ms],
                                         rhs=y_sb[:, ch, qs],
                                         start=(ch == 0), stop=(ch == 1))
                    if mt % 2 == 0:
                        nc.vector.tensor_copy(ot[:, mt, :], pp)
                    else:
                        nc.scalar.activation(ot[:, mt, :], pp, AF.Copy)
                nc.sync.dma_start(out=outT_p[:, :, qs], in_=ot)

            # causal phase order, ending with the light qt0 attention so
            # the big qt3 output DMA hides under compute.  Independent PE
            # work (qkv pairs, v chunks, deferred proj) is spliced between
            # attention phases to cover RoPE latency and psum-pool drains.
            emit_qkv(0)
            for tt in range(0, 4):
                emit_v(tt)
            emit_qkv(1)
            for tt in range(4, 8):
                emit_v(tt)
            emit_qkv(2)
            emit_attn(0, 1)
            emit_attn(1, 1)
            for tt in range(8, 12):
                emit_v(tt)
            emit_proj(1)
            emit_attn(0, 2)
            emit_qkv(3)
            emit_attn(1, 2)
            for tt in range(12, 16):
                emit_v(tt)
            emit_attn(0, 3)
            emit_proj(2)
            emit_attn(1, 3)
            emit_proj(3)
            emit_attn(0, 0)
            emit_attn(1, 0)
            emit_proj(0)

    nc.finalize()
    return nc


def _prep_inputs(x, cos, sin, w_qkv, w_proj):
    sq, am_q = _ternarize_host(w_qkv)
    sp, am_p = _ternarize_host(w_proj)

    # head-dim row order: interleave (d, d+32) so rotate-half is a
    # swap-adjacent-rows shuffle
    perm = np.empty(D, dtype=np.int64)
    perm[0::2] = np.arange(32)
    perm[1::2] = np.arange(32, 64)

    cos_t = np.ascontiguousarray(cos[0, 0].T).astype(np.float32)  # [D, T]
    sin_t = np.ascontiguousarray(sin[0, 0].T).astype(np.float32)
    sgn = np.where(np.arange(D) < 32, np.float32(-1.0), np.float32(1.0))
    ss_t = sin_t * sgn[:, None]
    cos2 = np.ascontiguousarray(cos_t[perm]).astype(BF16)   # [64, T]
    ss2 = np.ascontiguousarray(ss_t[perm]).astype(BF16)
    csss = np.empty((P, 2, T), dtype=BF16)
    csss[0:64, 0] = cos2
    csss[64:128, 0] = cos2
    csss[0:64, 1] = ss2
    csss[64:128, 1] = ss2

    in_maps = []
    for core in range(N_CORES):
        b, g = divmod(core, HEADS_PER_CORE)
        heads = [4 * g + h for h in range(4)]
        q_rows = np.concatenate([h * D + perm for h in heads])
        k_rows = C + q_rows
        v_rows_n = np.concatenate(
            [np.arange(h * D, (h + 1) * D) for h in heads])
        v_rows = 2 * C + v_rows_n
        wqk_block = np.concatenate([sq[q_rows], sq[k_rows]], axis=0)
        wqk_t = np.ascontiguousarray(wqk_block.T).astype(F8E4)   # [C, 512]
        wv_t = np.ascontiguousarray(sq[v_rows].T).astype(BF16)   # [C, 256]
        wp_t = np.ascontiguousarray(sp[:, v_rows_n].T).astype(BF16)  # [256, C]
        xt = np.ascontiguousarray(x[b].T).astype(BF16)           # [C, T]
        xq = xt.astype(F8E4)
        in_maps.append({
            "xb": xt, "xq8": xq, "wqk": wqk_t, "wv": wv_t, "wp": wp_t,
            "csss": csss,
        })
    exp_scale = float(am_q) * float(am_q) / float(np.sqrt(np.float32(D)))
    return in_maps, np.float32(am_q * am_p), exp_scale


def kernel(x, cos, sin, w_qkv, w_proj):
    x = np.asarray(x, dtype=np.float32)
    cos = np.asarray(cos, dtype=np.float32)
    sin = np.asarray(sin, dtype=np.float32)
    w_qkv = np.asarray(w_qkv, dtype=np.float32)
    w_proj = np.asarray(w_proj, dtype=np.float32)

    _install_ntff_hook()
    from concourse.bass_utils import run_bass_kernel_spmd

    in_maps, out_scale, exp_scale = _prep_inputs(x, cos, sin, w_qkv, w_proj)
    if "nc" not in _CACHE:
        _CACHE["nc"] = _build_program(exp_scale)
    nc = _CACHE["nc"]
    trace = bool(os.environ.get("KERNEL_TRACE"))
    res = run_bass_kernel_spmd(nc, in_maps, core_ids=list(range(N_CORES)),
                               trace=trace)
    _CACHE["exec_time_ns"] = res.exec_time_ns

    out = np.zeros((B, T, C), dtype=np.float32)
    for core in range(N_CORES):
        b = core // HEADS_PER_CORE
        out[b] += res.results[core]["outT"].astype(np.float32).T
    out *= out_scale
    return out


# revision 11
# speedup vs baseline: 1.0283x; 1.0283x over previous
"""Trainium2 Bass kernel for nn_Attention_18786186952997.

Dense causal-attention transformer block with ternarized (BitNet-style)
weights and RoPE:

    wq = ternarize(w_qkv); wp = ternarize(w_proj)
    qkv = x @ wq.T ; q,k,v split ; RoPE(q,k) ; causal SDPA ; y @ wp.T

Sharding: 8 cores = 2 batches x 4 head-groups (4 heads each).  Each core
computes its batch's qkv projections for its 4 heads, runs causal
flash-style attention fully on-chip, and produces a partial (transposed)
projection output; the host sums the 4 partials per batch (scaled by
am_q*am_p on the host, so the device works with pure sign weights).

Device compute layout is channel-major: q.T/k.T are produced as
[head_dim, tokens].  The head-dim rows are interleaved (d, d+32) pairs so
RoPE's rotate-half becomes a swap-adjacent-rows stream_shuffle (one DVE
op) instead of four 32-row shifted multiplies.  exp(scores.T) is exactly
the stationary layout A@V needs; softmax denominators come free from
ones-columns packed next to V.

q/k projections run on single-precision fp8 x with DoubleRow packing a
PAIR of 128-channel contraction chunks per matmul (half the matmuls of
the residual-pair scheme; ~7e-3 rel err, sim-validated).  v projections
run on bf16 x without DoubleRow so FWL keeps LDWEIGHTS off the critical
path.  Scores/AV/proj stay bf16.  Dummy matmuls at t=0 warm the PE HAM
clock gate while the first DMAs land.  Causal masking affine_selects
only the 128-column diagonal band.  Phases run in causal order with
independent PE work spliced between attention phases.
"""

import os
import sys
import types

import numpy as np

sys.path.insert(0, "/opt/trn_rl_repo")

import ml_dtypes  # noqa: E402

BF16 = ml_dtypes.bfloat16
F8E4 = ml_dtypes.float8_e4m3

B, T, C, H, D = 2, 2048, 1024, 16, 64
N_CORES = 8
HEADS_PER_CORE = 4
P = 128
QT = 512            # q tile (moving free dim)
NQT = T // QT       # 4
NKC = T // P        # 16 k chunks
NCC = C // P        # 8 contraction chunks

_CACHE = {}


def _install_ntff_hook():
    """bass_utils' trace=True path needs antenv.axon_hooks, absent in this
    image; synthesize it around the boot module's ctypes hook."""
    if "antenv.axon_hooks" in sys.modules:
        return
    try:
        import antenv  # noqa: F401
        from trn_agent_boot.trn_boot import _ntff_profile_via_ctypes
    except Exception:
        return
    mod = types.ModuleType("antenv.axon_hooks")
    holder = {}
    mod.set_axon_ntff_profile_hook = lambda h: holder.__setitem__("h", h)
    mod.get_axon_ntff_profile_hook = lambda: holder.get("h")
    sys.modules["antenv.axon_hooks"] = mod
    sys.modules["antenv"].axon_hooks = mod
    try:
        hook = _ntff_profile_via_ctypes("/opt/axon/libaxon_pjrt.so")
        mod.set_axon_ntff_profile_hook(hook)
    except Exception:
        pass


def _ternarize_host(w):
    """Sign matrix and abs-mean scale, bit-matching the jax reference."""
    try:
        import jax.numpy as jnp

        wj = jnp.asarray(w)
        am = jnp.maximum(jnp.abs(wj).mean(), 1e-5)
        thr = 0.7 * am
        s = jnp.where(wj > thr, 1.0, jnp.where(wj < -thr, -1.0, 0.0))
        return np.asarray(s, dtype=np.float32), np.float32(am)
    except Exception:
        am = np.float32(max(np.abs(w).astype(np.float32).mean(dtype=np.float32), 1e-5))
        thr = np.float32(0.7) * am
        s = np.where(w > thr, 1.0, np.where(w < -thr, -1.0, 0.0)).astype(np.float32)
        return s, am


def _build_program(exp_scale):
    import concourse.bass as bass  # noqa: F401
    import concourse.mybir as mybir
    import concourse.tile as tile
    from concourse import bacc

    F32 = mybir.dt.float32
    BF = mybir.dt.bfloat16
    F8 = mybir.dt.float8e4
    AF = mybir.ActivationFunctionType
    GE = mybir.AluOpType.is_ge
    DR = mybir.MatmulPerfMode.DoubleRow

    nc = bacc.Bacc("TRN2", target_bir_lowering=False, debug=False,
                   num_devices=N_CORES)

    # bf16 x is the only x upload; the fp8 copy for q/k (DoubleRow packs
    # kc pairs) is cast on-chip.  Ternary weights exact in fp8/bf16.
    xb = nc.dram_tensor("xb", [C, T], BF, kind="ExternalInput").ap()
    wqk = nc.dram_tensor("wqk", [C, 512], F8, kind="ExternalInput").ap()
    wv = nc.dram_tensor("wv", [C, 256], BF, kind="ExternalInput").ap()
    wp = nc.dram_tensor("wp", [256, 1024], BF, kind="ExternalInput").ap()
    csss = nc.dram_tensor("csss", [P, 2, T], BF, kind="ExternalInput").ap()
    outT = nc.dram_tensor("outT", [C, T], BF, kind="ExternalOutput").ap()

    # swap-adjacent-rows shuffle mask (per 32-partition quadrant)
    swap_mask = []
    for i in range(16):
        swap_mask += [2 * i + 1, 2 * i]

    with tile.TileContext(nc) as tc:
        with (
            tc.tile_pool(name="consts", bufs=1) as consts,
            tc.tile_pool(name="tmps", bufs=3) as tmps,
            tc.tile_pool(name="epool", bufs=6) as epool,
            tc.tile_pool(name="opool", bufs=2) as opool,
            tc.tile_pool(name="ps_big", bufs=3, space="PSUM") as ps_big,
            tc.tile_pool(name="ps_y", bufs=2, space="PSUM") as ps_y,
        ):
            # ---- persistent SBUF allocations ----
            cs_sb = consts.tile([P, 2, T], BF)          # cos | signed-sin
            x_sb = consts.tile([P, NCC, T], BF)         # bf16 x (for v)
            xq_sb = consts.tile([P, 4, 2, T], F8)       # fp8 x (for q/k)
            wqk_sb = consts.tile([P, 4, 2, 512], F8)
            wv_sb = consts.tile([P, NCC, 256], BF)
            wp_sb = consts.tile([P, 2, 1024], BF)
            qk_sb = consts.tile([P, 4, T], BF)  # blk: q01, q23, k01, k23
            v_sb = consts.tile([P, NKC, 2, 256], BF)
            y_sb = consts.tile([P, 2, T], BF)
            warm_sb = consts.tile([P, 512], BF)

            xb_p = xb.rearrange("(n p) t -> p n t", p=P)
            wqk_p = wqk.rearrange("(n r p) m -> p n r m", p=P, r=2)
            wv_p = wv.rearrange("(n p) m -> p n m", p=P)
            wp_p = wp.rearrange("(n p) m -> p n m", p=P)
            outT_p = outT.rearrange("(m p) t -> p m t", p=P)
            # fp8 x viewed with the kc-pair axis flattened: same memory
            # order as x_sb, so the on-chip cast is a straight copy
            xq_v = xq_sb.rearrange("p n r t -> p (n r) t")

            # ---- PE warm-up: ~5us of dummy matmuls releases the HAM
            # clock gate while the first input DMAs are still landing ----
            nc.vector.memset(warm_sb, 0.0)
            wps = ps_y.tile([P, 512], F32, tag="y", name="warm")
            for i in range(11):
                nc.tensor.matmul(wps, lhsT=warm_sb[:, 0:P], rhs=warm_sb,
                                 start=(i == 0), stop=(i == 10))

            # ---- DMA-in, ordered by first use; x split per-qt into kc
            # halves across two parallel queues (sync + gpsimd), weights
            # on the scalar queue (issued before exp work begins) ----
            nc.scalar.dma_start(out=wqk_sb, in_=wqk_p)
            for qt in range(NQT):
                qs = slice(qt * QT, (qt + 1) * QT)
                nc.sync.dma_start(out=x_sb[:, 0:4, qs], in_=xb_p[:, 0:4, qs])
                nc.gpsimd.dma_start(out=x_sb[:, 4:8, qs],
                                    in_=xb_p[:, 4:8, qs])
                if qt == 0:
                    nc.scalar.dma_start(out=wv_sb, in_=wv_p)
                    nc.scalar.dma_start(out=cs_sb[:, :, 0:QT],
                                        in_=csss[:, :, 0:QT])
                if qt == 2:
                    nc.scalar.dma_start(out=cs_sb[:, :, QT:],
                                        in_=csss[:, :, QT:])
            nc.scalar.dma_start(out=wp_sb, in_=wp_p)

            def emit_cast(qt):
                qs = slice(qt * QT, (qt + 1) * QT)
                nc.vector.tensor_copy(xq_v[:, :, qs], x_sb[:, :, qs])

            # per head: [ones(64) | v(64)] -> denominators at psum rows 0:64
            v_sb4 = v_sb.rearrange("p n g (h o d) -> p n g h o d", h=2, o=2)
            nc.gpsimd.memset(v_sb4[:, :, :, :, 0, :], 1.0)

            def emit_qkv(qt):
                # wqk col blocks: QA[0:256) KA[256:512)
                qs = slice(qt * QT, (qt + 1) * QT)
                for pair in range(2):
                    blks = ((0, 2), (1, 3))[pair]
                    ps = ps_big.tile([P, 1024], F32, tag="big", name="qkvps")
                    for kcp in range(4):
                        for j, blk in enumerate(blks):
                            base_a = [0, 128, 256, 384][blk]
                            nc.tensor.matmul(
                                ps[:, j * QT:(j + 1) * QT],
                                lhsT=wqk_sb[:, kcp, :, base_a:base_a + P],
                                rhs=xq_sb[:, kcp, :, qs],
                                start=(kcp == 0),
                                stop=(kcp == 3),
                                perf_mode=DR,
                            )
                    for j, blk in enumerate(blks):
                        pj = ps[:, j * QT:(j + 1) * QT]
                        # rows interleaved (d, d+32): rotate-half = swap pairs
                        t2s = tmps.tile([P, QT], F32, tag="t2s")
                        nc.vector.stream_shuffle(t2s, pj, swap_mask)
                        t1 = tmps.tile([P, QT], F32, tag="t1")
                        nc.vector.tensor_mul(t1, pj, cs_sb[:, 0, qs])
                        t2 = tmps.tile([P, QT], F32, tag="t2")
                        nc.gpsimd.tensor_mul(t2, t2s, cs_sb[:, 1, qs])
                        nc.gpsimd.tensor_add(qk_sb[:, blk, qs], t1, t2)

            def emit_v(tt):
                vp = ps_y.tile([P, 512], F32, tag="y", name="vps")
                for kc in range(NCC):
                    nc.tensor.matmul(
                        vp[:, 0:256],
                        lhsT=x_sb[:, kc, tt * P:(tt + 1) * P],
                        rhs=wv_sb[:, kc, :],
                        start=(kc == 0),
                        stop=(kc == NCC - 1),
                    )
                vp4 = vp[:, 0:256].rearrange("p (g h d) -> p g h d", g=2, h=2)
                nc.vector.tensor_copy(v_sb4[:, tt, :, :, 1, :], vp4)

            def emit_attn(grp, qt):
                q_t = qk_sb[:, grp, :]
                k_t = qk_sb[:, 2 + grp, :]
                qs = slice(qt * QT, (qt + 1) * QT)
                KC = 4 * (qt + 1)  # causal k chunks
                yA = ps_y.tile([P, QT], F32, tag="y", name="yA")
                yB = ps_y.tile([P, QT], F32, tag="y", name="yB")
                es = [None] * KC

                def emit_sc(kc):
                    ks = slice(kc * P, (kc + 1) * P)
                    delta = max(kc * P - qt * QT, 0)
                    # queries < delta cannot see this key chunk: compute
                    # scores/exp only on the [delta:QT) query slice
                    qsl = slice(qt * QT + delta, (qt + 1) * QT)
                    ps = ps_big.tile([P, 1024], F32, tag="big", name="scps")
                    p2 = ps.rearrange("p (j f) -> p j f", j=2)
                    e = epool.tile([P, 1024], BF, tag="e")
                    e2 = e.rearrange("p (j f) -> p j f", j=2)
                    nc.tensor.matmul(p2[:, 0, delta:QT], lhsT=k_t[0:64, ks],
                                     rhs=q_t[0:64, qsl],
                                     start=True, stop=True)
                    nc.tensor.matmul(p2[:, 1, delta:QT], lhsT=k_t[64:128, ks],
                                     rhs=q_t[64:128, qsl],
                                     start=True, stop=True)
                    nc.scalar.activation(e2[:, :, delta:QT],
                                         p2[:, :, delta:QT],
                                         AF.Exp, scale=exp_scale)
                    if kc * P >= qt * QT:
                        # diagonal chunk: zero keys below the diagonal for
                        # both heads in one op (iota = col' - p >= 0); only
                        # the first 128 query columns past delta are mixed
                        nc.gpsimd.affine_select(
                            e2[:, :, delta:delta + P], e2[:, :, delta:delta + P],
                            pattern=[[0, 2], [1, P]],
                            compare_op=GE, fill=0.0,
                            base=0, channel_multiplier=-1)
                    es[kc] = (e2, delta)

                def emit_av(kc):
                    e2, delta = es[kc]
                    nc.tensor.matmul(yA[:, delta:QT],
                                     lhsT=v_sb[:, kc, grp, 0:128],
                                     rhs=e2[:, 0, delta:QT],
                                     start=(kc == 0), stop=(kc == KC - 1),
                                     skip_group_check=True)
                    nc.tensor.matmul(yB[:, delta:QT],
                                     lhsT=v_sb[:, kc, grp, 128:256],
                                     rhs=e2[:, 1, delta:QT],
                                     start=(kc == 0), stop=(kc == KC - 1),
                                     skip_group_check=True)

                # depth-4 software pipeline: AV lags scores by 4 chunks
                LAG = min(4, KC - 1)
                for kc in range(LAG):
                    emit_sc(kc)
                for kc in range(LAG, KC):
                    emit_sc(kc)
                    emit_av(kc - LAG)
                for kc in range(KC - LAG, KC):
                    emit_av(kc)

                # both heads: denom rows 0:64, y rows 64:128
                rcA = tmps.tile([P, QT], F32, tag="rc")
                nc.vector.reciprocal_approx_fast(rcA[0:64, :], yA[0:64, :])
                nc.vector.tensor_mul(y_sb[0:64, grp, qs], yA[64:128, :],
                                     rcA[0:64, :])
                rcB = tmps.tile([P, QT], F32, tag="rc")
                nc.vector.reciprocal_approx_fast(rcB[0:64, :], yB[0:64, :])
                nc.vector.tensor_mul(y_sb[64:128, grp, qs], yB[64:128, :],
                                     rcB[0:64, :])

            def emit_proj(qt):
                qs = slice(qt * QT, (qt + 1) * QT)
                ot = opool.tile([P, 8, QT], BF, tag="ot")
                for mt in range(8):
                    ms = slice(mt * P, (mt + 1) * P)
                    pp = ps_y.tile([P, QT], F32, tag="y", name="pp")
                    for ch in range(2):
                        nc.tensor.matmul(pp, lhsT=wp_sb[:, ch, ms],
                                         rhs=y_sb[:, ch, qs],
                                         start=(ch == 0), stop=(ch == 1))
                    nc.vector.tensor_copy(ot[:, mt, :], pp)
                if qt == 0:
                    # final output tile: split across both queues so the
                    # tail drain halves
                    nc.sync.dma_start(out=outT_p[:, 0:4, qs],
                                      in_=ot[:, 0:4, :])
                    nc.gpsimd.dma_start(out=outT_p[:, 4:8, qs],
                                        in_=ot[:, 4:8, :])
                else:
                    nc.sync.dma_start(out=outT_p[:, :, qs], in_=ot)

            # causal phase order, ending with the light qt0 attention so
            # the big qt3 output DMA hides under compute.  Independent PE
            # work (qkv pairs, v chunks, deferred proj) is spliced between
            # attention phases to cover RoPE latency and psum-pool drains.
            emit_cast(0)
            emit_cast(1)
            emit_qkv(0)
            for tt in range(0, 4):
                emit_v(tt)
            emit_qkv(1)
            emit_cast(2)
            for tt in range(4, 8):
                emit_v(tt)
            emit_qkv(2)
            emit_cast(3)
            emit_attn(0, 1)
            emit_attn(1, 1)
            for tt in range(8, 12):
                emit_v(tt)
            emit_proj(1)
            emit_attn(0, 2)
            emit_qkv(3)
            emit_attn(1, 2)
            for tt in range(12, 16):
                emit_v(tt)
            emit_attn(0, 3)
            emit_proj(2)
            emit_attn(1, 3)
            emit_proj(3)
            emit_attn(0, 0)
            emit_attn(1, 0)
            emit_proj(0)

    nc.finalize()
    return nc


def _prep_inputs(x, cos, sin, w_qkv, w_proj):
    sq, am_q = _ternarize_host(w_qkv)
    sp, am_p = _ternarize_host(w_proj)

    # head-dim row order: interleave (d, d+32) so rotate-half is a
    # swap-adjacent-rows shuffle
    perm = np.empty(D, dtype=np.int64)
    perm[0::2] = np.arange(32)
    perm[1::2] = np.arange(32, 64)

    cos_t = np.ascontiguousarray(cos[0, 0].T).astype(np.float32)  # [D, T]
    sin_t = np.ascontiguousarray(sin[0, 0].T).astype(np.float32)
    sgn = np.where(np.arange(D) < 32, np.float32(-1.0), np.float32(1.0))
    ss_t = sin_t * sgn[:, None]
    cos2 = np.ascontiguousarray(cos_t[perm]).astype(BF16)   # [64, T]
    ss2 = np.ascontiguousarray(ss_t[perm]).astype(BF16)
    csss = np.empty((P, 2, T), dtype=BF16)
    csss[0:64, 0] = cos2
    csss[64:128, 0] = cos2
    csss[0:64, 1] = ss2
    csss[64:128, 1] = ss2

    in_maps = []
    for core in range(N_CORES):
        b, g = divmod(core, HEADS_PER_CORE)
        heads = [4 * g + h for h in range(4)]
        q_rows = np.concatenate([h * D + perm for h in heads])
        k_rows = C + q_rows
        v_rows_n = np.concatenate(
            [np.arange(h * D, (h + 1) * D) for h in heads])
        v_rows = 2 * C + v_rows_n
        wqk_block = np.concatenate([sq[q_rows], sq[k_rows]], axis=0)
        wqk_t = np.ascontiguousarray(wqk_block.T).astype(F8E4)   # [C, 512]
        wv_t = np.ascontiguousarray(sq[v_rows].T).astype(BF16)   # [C, 256]
        wp_t = np.ascontiguousarray(sp[:, v_rows_n].T).astype(BF16)  # [256, C]
        xt = np.ascontiguousarray(x[b].T).astype(BF16)           # [C, T]
        in_maps.append({
            "xb": xt, "wqk": wqk_t, "wv": wv_t, "wp": wp_t,
            "csss": csss,
        })
    exp_scale = float(am_q) * float(am_q) / float(np.sqrt(np.float32(D)))
    return in_maps, np.float32(am_q * am_p), exp_scale


def kernel(x, cos, sin, w_qkv, w_proj):
    x = np.asarray(x, dtype=np.float32)
    cos = np.asarray(cos, dtype=np.float32)
    sin = np.asarray(sin, dtype=np.float32)
    w_qkv = np.asarray(w_qkv, dtype=np.float32)
    w_proj = np.asarray(w_proj, dtype=np.float32)

    _install_ntff_hook()
    from concourse.bass_utils import run_bass_kernel_spmd

    in_maps, out_scale, exp_scale = _prep_inputs(x, cos, sin, w_qkv, w_proj)
    if "nc" not in _CACHE:
        _CACHE["nc"] = _build_program(exp_scale)
    nc = _CACHE["nc"]
    trace = bool(os.environ.get("KERNEL_TRACE"))
    res = run_bass_kernel_spmd(nc, in_maps, core_ids=list(range(N_CORES)),
                               trace=trace)
    _CACHE["exec_time_ns"] = res.exec_time_ns

    out = np.zeros((B, T, C), dtype=np.float32)
    for core in range(N_CORES):
        b = core // HEADS_PER_CORE
        out[b] += res.results[core]["outT"].astype(np.float32).T
    out *= out_scale
    return out


# revision 15
# speedup vs baseline: 1.0573x; 1.0282x over previous
"""Trainium2 Bass kernel for nn_Attention_18786186952997.

Dense causal-attention transformer block with ternarized (BitNet-style)
weights and RoPE:

    wq = ternarize(w_qkv); wp = ternarize(w_proj)
    qkv = x @ wq.T ; q,k,v split ; RoPE(q,k) ; causal SDPA ; y @ wp.T

Sharding: 8 cores = 2 batches x 4 head-groups (4 heads each).  Each core
computes its batch's qkv projections for its 4 heads, runs causal
flash-style attention fully on-chip, and produces a partial (transposed)
projection output; the host sums the 4 partials per batch (scaled by
am_q*am_p on the host, so the device works with pure sign weights).

Device compute layout is channel-major: q.T/k.T are produced as
[head_dim, tokens].  The head-dim rows are interleaved (d, d+32) pairs so
RoPE's rotate-half becomes a swap-adjacent-rows stream_shuffle (one DVE
op) instead of four 32-row shifted multiplies.  exp(scores.T) is exactly
the stationary layout A@V needs; softmax denominators come free from
ones-columns packed next to V.

q/k projections run on single-precision fp8 x with DoubleRow packing a
PAIR of 128-channel contraction chunks per matmul (half the matmuls of
the residual-pair scheme; ~7e-3 rel err, sim-validated).  v projections
run on bf16 x without DoubleRow so FWL keeps LDWEIGHTS off the critical
path.  Scores/AV/proj stay bf16.  Dummy matmuls at t=0 warm the PE HAM
clock gate while the first DMAs land.  Causal masking affine_selects
only the 128-column diagonal band.  Phases run in causal order with
independent PE work spliced between attention phases.
"""

import os
import sys
import types

import numpy as np

sys.path.insert(0, "/opt/trn_rl_repo")

import ml_dtypes  # noqa: E402

BF16 = ml_dtypes.bfloat16
F8E4 = ml_dtypes.float8_e4m3

B, T, C, H, D = 2, 2048, 1024, 16, 64
N_CORES = 8
HEADS_PER_CORE = 4
P = 128
QT = 512            # q tile (moving free dim)
NQT = T // QT       # 4
NKC = T // P        # 16 k chunks
NCC = C // P        # 8 contraction chunks

_CACHE = {}


def _install_ntff_hook():
    """bass_utils' trace=True path needs antenv.axon_hooks, absent in this
    image; synthesize it around the boot module's ctypes hook."""
    if "antenv.axon_hooks" in sys.modules:
        return
    try:
        import antenv  # noqa: F401
        from trn_agent_boot.trn_boot import _ntff_profile_via_ctypes
    except Exception:
        return
    mod = types.ModuleType("antenv.axon_hooks")
    holder = {}
    mod.set_axon_ntff_profile_hook = lambda h: holder.__setitem__("h", h)
    mod.get_axon_ntff_profile_hook = lambda: holder.get("h")
    sys.modules["antenv.axon_hooks"] = mod
    sys.modules["antenv"].axon_hooks = mod
    try:
        hook = _ntff_profile_via_ctypes("/opt/axon/libaxon_pjrt.so")
        mod.set_axon_ntff_profile_hook(hook)
    except Exception:
        pass


def _ternarize_host(w):
    """Sign matrix and abs-mean scale, bit-matching the jax reference."""
    try:
        import jax.numpy as jnp

        wj = jnp.asarray(w)
        am = jnp.maximum(jnp.abs(wj).mean(), 1e-5)
        thr = 0.7 * am
        s = jnp.where(wj > thr, 1.0, jnp.where(wj < -thr, -1.0, 0.0))
        return np.asarray(s, dtype=np.float32), np.float32(am)
    except Exception:
        am = np.float32(max(np.abs(w).astype(np.float32).mean(dtype=np.float32), 1e-5))
        thr = np.float32(0.7) * am
        s = np.where(w > thr, 1.0, np.where(w < -thr, -1.0, 0.0)).astype(np.float32)
        return s, am


def _build_program(exp_scale):
    import concourse.bass as bass  # noqa: F401
    import concourse.mybir as mybir
    import concourse.tile as tile
    from concourse import bacc

    F32 = mybir.dt.float32
    BF = mybir.dt.bfloat16
    F8 = mybir.dt.float8e4
    AF = mybir.ActivationFunctionType
    GE = mybir.AluOpType.is_ge
    DR = mybir.MatmulPerfMode.DoubleRow

    nc = bacc.Bacc("TRN2", target_bir_lowering=False, debug=False,
                   num_devices=N_CORES)

    # bf16 x is the only x upload; the fp8 copy for q/k (DoubleRow packs
    # kc pairs) is cast on-chip.  Ternary weights exact in fp8/bf16.
    xb = nc.dram_tensor("xb", [C, T], BF, kind="ExternalInput").ap()
    wqk = nc.dram_tensor("wqk", [C, 512], F8, kind="ExternalInput").ap()
    wv = nc.dram_tensor("wv", [C, 256], BF, kind="ExternalInput").ap()
    wp = nc.dram_tensor("wp", [256, 1024], BF, kind="ExternalInput").ap()
    csss = nc.dram_tensor("csss", [P, 2, T], BF, kind="ExternalInput").ap()
    outT = nc.dram_tensor("outT", [C, T], BF, kind="ExternalOutput").ap()

    # swap-adjacent-rows shuffle mask (per 32-partition quadrant)
    swap_mask = []
    for i in range(16):
        swap_mask += [2 * i + 1, 2 * i]

    with tile.TileContext(nc) as tc:
        with (
            tc.tile_pool(name="consts", bufs=1) as consts,
            tc.tile_pool(name="tmps", bufs=3) as tmps,
            tc.tile_pool(name="epool", bufs=6) as epool,
            tc.tile_pool(name="opool", bufs=2) as opool,
            tc.tile_pool(name="ps_big", bufs=3, space="PSUM") as ps_big,
            tc.tile_pool(name="ps_y", bufs=2, space="PSUM") as ps_y,
        ):
            # ---- persistent SBUF allocations ----
            cs_sb = consts.tile([P, 2, T], BF)          # cos | signed-sin
            x_sb = consts.tile([P, NCC, T], BF)         # bf16 x (for v)
            xq_sb = consts.tile([P, 4, 2, T], F8)       # fp8 x (for q/k)
            wqk_sb = consts.tile([P, 4, 2, 512], F8)
            wv_sb = consts.tile([P, NCC, 256], BF)
            wp_sb = consts.tile([P, 2, 1024], BF)
            qk_sb = consts.tile([P, 4, T], BF)  # blk: q01, q23, k01, k23
            v_sb = consts.tile([P, NKC, 2, 256], BF)
            y_sb = consts.tile([P, 2, T], BF)
            warm_sb = consts.tile([P, 512], BF)

            xb_p = xb.rearrange("(n p) t -> p n t", p=P)
            wqk_p = wqk.rearrange("(n r p) m -> p n r m", p=P, r=2)
            wv_p = wv.rearrange("(n p) m -> p n m", p=P)
            wp_p = wp.rearrange("(n p) m -> p n m", p=P)
            outT_p = outT.rearrange("(m p) t -> p m t", p=P)
            # fp8 x viewed with the kc-pair axis flattened: same memory
            # order as x_sb, so the on-chip cast is a straight copy
            xq_v = xq_sb.rearrange("p n r t -> p (n r) t")

            # ---- PE warm-up: ~5us of dummy matmuls releases the HAM
            # clock gate while the first input DMAs are still landing ----
            nc.vector.memset(warm_sb, 0.0)
            wps = ps_y.tile([P, 512], F32, tag="y", name="warm")
            for i in range(11):
                nc.tensor.matmul(wps, lhsT=warm_sb[:, 0:P], rhs=warm_sb,
                                 start=(i == 0), stop=(i == 10))

            # ---- DMA-in, ordered by first use; x split per-qt into kc
            # halves across two parallel queues (sync + gpsimd), weights
            # on the scalar queue (issued before exp work begins).  The
            # first tranche (xb-qt0 + wv + wqk) is exclusive so the first
            # PE work can start ~13us ----
            for qt in range(NQT):
                qs = slice(qt * QT, (qt + 1) * QT)
                nc.sync.dma_start(out=x_sb[:, 0:4, qs], in_=xb_p[:, 0:4, qs])
                nc.gpsimd.dma_start(out=x_sb[:, 4:8, qs],
                                    in_=xb_p[:, 4:8, qs])
                if qt == 0:
                    nc.scalar.dma_start(out=wv_sb, in_=wv_p)
                if qt == 1:
                    nc.scalar.dma_start(out=wqk_sb, in_=wqk_p)
                    nc.scalar.dma_start(out=cs_sb[:, :, 0:QT],
                                        in_=csss[:, :, 0:QT])
                if qt == 2:
                    nc.scalar.dma_start(out=cs_sb[:, :, QT:],
                                        in_=csss[:, :, QT:])
            nc.scalar.dma_start(out=wp_sb, in_=wp_p)

            def emit_cast(qt):
                qs = slice(qt * QT, (qt + 1) * QT)
                nc.vector.tensor_copy(xq_v[:, :, qs], x_sb[:, :, qs])

            # per head: [ones(64) | v(64)] -> denominators at psum rows 0:64
            v_sb4 = v_sb.rearrange("p n g (h o d) -> p n g h o d", h=2, o=2)
            nc.gpsimd.memset(v_sb4[:, :, :, :, 0, :], 1.0)

            def emit_qkv(qt):
                # wqk col blocks: QA[0:256) KA[256:512)
                qs = slice(qt * QT, (qt + 1) * QT)
                for pair in range(2):
                    blks = ((0, 2), (1, 3))[pair]
                    ps = ps_big.tile([P, 1024], F32, tag="big", name="qkvps")
                    for kcp in range(4):
                        for j, blk in enumerate(blks):
                            base_a = [0, 128, 256, 384][blk]
                            nc.tensor.matmul(
                                ps[:, j * QT:(j + 1) * QT],
                                lhsT=wqk_sb[:, kcp, :, base_a:base_a + P],
                                rhs=xq_sb[:, kcp, :, qs],
                                start=(kcp == 0),
                                stop=(kcp == 3),
                                perf_mode=DR,
                            )
                    for j, blk in enumerate(blks):
                        pj = ps[:, j * QT:(j + 1) * QT]
                        # rows interleaved (d, d+32): rotate-half = swap
                        # pairs.  bf16 intermediates give 16-bit 2x modes
                        # on both DVE and gpsimd.
                        qb = tmps.tile([P, QT], BF, tag="qb")
                        nc.vector.tensor_copy(qb, pj)
                        t2s = tmps.tile([P, QT], BF, tag="t2s")
                        nc.vector.stream_shuffle(t2s, qb, swap_mask)
                        t1 = tmps.tile([P, QT], BF, tag="t1")
                        nc.vector.tensor_mul(t1, qb, cs_sb[:, 0, qs])
                        t2 = tmps.tile([P, QT], BF, tag="t2")
                        nc.gpsimd.tensor_mul(t2, t2s, cs_sb[:, 1, qs])
                        nc.gpsimd.tensor_add(qk_sb[:, blk, qs], t1, t2)

            def emit_v(tt):
                vp = ps_y.tile([P, 512], F32, tag="y", name="vps")
                for kc in range(NCC):
                    nc.tensor.matmul(
                        vp[:, 0:256],
                        lhsT=x_sb[:, kc, tt * P:(tt + 1) * P],
                        rhs=wv_sb[:, kc, :],
                        start=(kc == 0),
                        stop=(kc == NCC - 1),
                    )
                vp4 = vp[:, 0:256].rearrange("p (g h d) -> p g h d", g=2, h=2)
                nc.vector.tensor_copy(v_sb4[:, tt, :, :, 1, :], vp4)

            def emit_attn(grp, qt):
                q_t = qk_sb[:, grp, :]
                k_t = qk_sb[:, 2 + grp, :]
                qs = slice(qt * QT, (qt + 1) * QT)
                KC = 4 * (qt + 1)  # causal k chunks
                yA = ps_y.tile([P, QT], F32, tag="y", name="yA")
                yB = ps_y.tile([P, QT], F32, tag="y", name="yB")
                es = [None] * KC

                def emit_sc(kc):
                    ks = slice(kc * P, (kc + 1) * P)
                    delta = max(kc * P - qt * QT, 0)
                    # queries < delta cannot see this key chunk: compute
                    # scores/exp only on the [delta:QT) query slice
                    qsl = slice(qt * QT + delta, (qt + 1) * QT)
                    ps = ps_big.tile([P, 1024], F32, tag="big", name="scps")
                    p2 = ps.rearrange("p (j f) -> p j f", j=2)
                    e = epool.tile([P, 1024], BF, tag="e")
                    e2 = e.rearrange("p (j f) -> p j f", j=2)
                    nc.tensor.matmul(p2[:, 0, delta:QT], lhsT=k_t[0:64, ks],
                                     rhs=q_t[0:64, qsl],
                                     start=True, stop=True)
                    nc.tensor.matmul(p2[:, 1, delta:QT], lhsT=k_t[64:128, ks],
                                     rhs=q_t[64:128, qsl],
                                     start=True, stop=True)
                    nc.scalar.activation(e2[:, :, delta:QT],
                                         p2[:, :, delta:QT],
                                         AF.Exp, scale=exp_scale)
                    if kc * P >= qt * QT:
                        # diagonal chunk: zero keys below the diagonal for
                        # both heads in one op (iota = col' - p >= 0); only
                        # the first 128 query columns past delta are mixed
                        nc.gpsimd.affine_select(
                            e2[:, :, delta:delta + P], e2[:, :, delta:delta + P],
                            pattern=[[0, 2], [1, P]],
                            compare_op=GE, fill=0.0,
                            base=0, channel_multiplier=-1)
                    es[kc] = (e2, delta)

                def emit_av(kc):
                    e2, delta = es[kc]
                    nc.tensor.matmul(yA[:, delta:QT],
                                     lhsT=v_sb[:, kc, grp, 0:128],
                                     rhs=e2[:, 0, delta:QT],
                                     start=(kc == 0), stop=(kc == KC - 1),
                                     skip_group_check=True)
                    nc.tensor.matmul(yB[:, delta:QT],
                                     lhsT=v_sb[:, kc, grp, 128:256],
                                     rhs=e2[:, 1, delta:QT],
                                     start=(kc == 0), stop=(kc == KC - 1),
                                     skip_group_check=True)

                # depth-4 software pipeline: AV lags scores by 4 chunks
                LAG = min(4, KC - 1)
                for kc in range(LAG):
                    emit_sc(kc)
                for kc in range(LAG, KC):
                    emit_sc(kc)
                    emit_av(kc - LAG)
                for kc in range(KC - LAG, KC):
                    emit_av(kc)

                # both heads: denom rows 0:64, y rows 64:128
                rcA = tmps.tile([P, QT], F32, tag="rc")
                nc.vector.reciprocal_approx_fast(rcA[0:64, :], yA[0:64, :])
                nc.vector.tensor_mul(y_sb[0:64, grp, qs], yA[64:128, :],
                                     rcA[0:64, :])
                rcB = tmps.tile([P, QT], F32, tag="rc")
                nc.vector.reciprocal_approx_fast(rcB[0:64, :], yB[0:64, :])
                nc.vector.tensor_mul(y_sb[64:128, grp, qs], yB[64:128, :],
                                     rcB[0:64, :])

            def emit_proj(qt):
                qs = slice(qt * QT, (qt + 1) * QT)
                ot = opool.tile([P, 8, QT], BF, tag="ot")
                for mt in range(8):
                    ms = slice(mt * P, (mt + 1) * P)
                    pp = ps_y.tile([P, QT], F32, tag="y", name="pp")
                    for ch in range(2):
                        nc.tensor.matmul(pp, lhsT=wp_sb[:, ch, ms],
                                         rhs=y_sb[:, ch, qs],
                                         start=(ch == 0), stop=(ch == 1))
                    nc.vector.tensor_copy(ot[:, mt, :], pp)
                if qt == 0:
                    # final output tile: split across both queues so the
                    # tail drain halves
                    nc.sync.dma_start(out=outT_p[:, 0:4, qs],
                                      in_=ot[:, 0:4, :])
                    nc.gpsimd.dma_start(out=outT_p[:, 4:8, qs],
                                        in_=ot[:, 4:8, :])
                else:
                    nc.sync.dma_start(out=outT_p[:, :, qs], in_=ot)

            # causal phase order, ending with the light qt0 attention so
            # the big qt3 output DMA hides under compute.  Independent PE
            # work (qkv pairs, v chunks, deferred proj) is spliced between
            # attention phases to cover RoPE latency and psum-pool drains.
            emit_cast(0)
            emit_cast(1)
            emit_qkv(0)
            for tt in range(0, 4):
                emit_v(tt)
            emit_qkv(1)
            emit_cast(2)
            for tt in range(4, 8):
                emit_v(tt)
            emit_attn(0, 1)
            emit_attn(1, 1)
            emit_qkv(2)
            for tt in range(8, 12):
                emit_v(tt)
            emit_proj(1)
            emit_attn(0, 2)
            emit_cast(3)
            emit_qkv(3)
            emit_attn(1, 2)
            for tt in range(12, 16):
                emit_v(tt)
            emit_attn(0, 3)
            emit_proj(2)
            emit_attn(1, 3)
            emit_proj(3)
            emit_attn(0, 0)
            emit_attn(1, 0)
            emit_proj(0)

    nc.finalize()
    return nc


def _prep_inputs(x, cos, sin, w_qkv, w_proj):
    sq, am_q = _ternarize_host(w_qkv)
    sp, am_p = _ternarize_host(w_proj)

    # head-dim row order: interleave (d, d+32) so rotate-half is a
    # swap-adjacent-rows shuffle
    perm = np.empty(D, dtype=np.int64)
    perm[0::2] = np.arange(32)
    perm[1::2] = np.arange(32, 64)

    cos_t = np.ascontiguousarray(cos[0, 0].T).astype(np.float32)  # [D, T]
    sin_t = np.ascontiguousarray(sin[0, 0].T).astype(np.float32)
    sgn = np.where(np.arange(D) < 32, np.float32(-1.0), np.float32(1.0))
    ss_t = sin_t * sgn[:, None]
    cos2 = np.ascontiguousarray(cos_t[perm]).astype(BF16)   # [64, T]
    ss2 = np.ascontiguousarray(ss_t[perm]).astype(BF16)
    csss = np.empty((P, 2, T), dtype=BF16)
    csss[0:64, 0] = cos2
    csss[64:128, 0] = cos2
    csss[0:64, 1] = ss2
    csss[64:128, 1] = ss2

    in_maps = []
    for core in range(N_CORES):
        b, g = divmod(core, HEADS_PER_CORE)
        heads = [4 * g + h for h in range(4)]
        q_rows = np.concatenate([h * D + perm for h in heads])
        k_rows = C + q_rows
        v_rows_n = np.concatenate(
            [np.arange(h * D, (h + 1) * D) for h in heads])
        v_rows = 2 * C + v_rows_n
        wqk_block = np.concatenate([sq[q_rows], sq[k_rows]], axis=0)
        wqk_t = np.ascontiguousarray(wqk_block.T).astype(F8E4)   # [C, 512]
        wv_t = np.ascontiguousarray(sq[v_rows].T).astype(BF16)   # [C, 256]
        wp_t = np.ascontiguousarray(sp[:, v_rows_n].T).astype(BF16)  # [256, C]
        xt = np.ascontiguousarray(x[b].T).astype(BF16)           # [C, T]
        in_maps.append({
            "xb": xt, "wqk": wqk_t, "wv": wv_t, "wp": wp_t,
            "csss": csss,
        })
    exp_scale = float(am_q) * float(am_q) / float(np.sqrt(np.float32(D)))
    return in_maps, np.float32(am_q * am_p), exp_scale


def kernel(x, cos, sin, w_qkv, w_proj):
    x = np.asarray(x, dtype=np.float32)
    cos = np.asarray(cos, dtype=np.float32)
    sin = np.asarray(sin, dtype=np.float32)
    w_qkv = np.asarray(w_qkv, dtype=np.float32)
    w_proj = np.asarray(w_proj, dtype=np.float32)

    _install_ntff_hook()
    from concourse.bass_utils import run_bass_kernel_spmd

    in_maps, out_scale, exp_scale = _prep_inputs(x, cos, sin, w_qkv, w_proj)
    if "nc" not in _CACHE:
        _CACHE["nc"] = _build_program(exp_scale)
    nc = _CACHE["nc"]
    trace = bool(os.environ.get("KERNEL_TRACE"))
    res = run_bass_kernel_spmd(nc, in_maps, core_ids=list(range(N_CORES)),
                               trace=trace)
    _CACHE["exec_time_ns"] = res.exec_time_ns

    out = np.zeros((B, T, C), dtype=np.float32)
    for core in range(N_CORES):
        b = core // HEADS_PER_CORE
        out[b] += res.results[core]["outT"].astype(np.float32).T
    out *= out_scale
    return out
